# revision 32
# baseline (speedup 1.0000x reference)
"""MoE SwiGLU (T=4096, D=I=1024, E=8, top-2) on 8 Trainium2 NeuronCores.

Expert-parallel with on-device routing, v7:

- Gate, hybrid: every core computes the fp32 gate for ITS 512 tokens
  (scores-major, natural expert order), publishes all-expert weights
  [E, 512] and a 16KB AllGather replicates them; each core then pulls its
  own expert's rows via an indirect row gather (per-core index input).
  Range 0 (tokens 0..1023) is ALSO gated locally with permuted gate
  weights (own expert = column 0), so phase C starts without waiting for
  the collective; a 4-byte dummy AllGather issued at t=0 absorbs the
  one-time CC bootstrap + launch skew while weights load.
- bf16 on the whole expert path; fp32 PSUM accumulate.
- Compaction entirely on PE+DVE (no DMA round trip, no queue hazards):
  matmul prefix-sums assign slots, then per 128-slot tile an fp16 one-hot
  (pos == slot) matmul accumulates each token's (tid_local, wgt, occ)
  payload into the compact list (fp16 keeps tid_local <= 1023 exact and
  its LDWEIGHTS at full rate).
- Phase C per range (4 x 1024 tokens): per c-tile indirect gather of
  routed x rows (pads redirected to the zero row), PE transposes to
  [D, tokens], SwiGLU, routing-weight scale, per c-tile indirect scatter
  into a zeroed [RT, D] bf16 contribution buffer (pad rows dropped by
  the DGE bounds check), bf16 ReduceScatter per range; shards ship to y
  at the very end (no mid-pipeline collective waits on any queue).
  Ranges are software-pipelined: compaction of q+1 goes before the
  w1/w3 stage of q, gathers+transposes of q+1 between w1/w3 and w2.
- Host-side pre-shuffled DRAM layouts keep big DMAs at >=4KB/descriptor.

Capacity: per (core, range) routed-token count for the fixed test seed is
256 +- 25 (max 281); CAP=288 with a host-side overflow check.
"""
import os
import sys

import numpy as np

for _p in ("/opt/trn_rl_repo", "/root/.axon_site/_ro/trn_rl_repo"):
    if os.path.isdir(_p) and _p not in sys.path:
        sys.path.append(_p)

import concourse.bass as bass  # noqa: E402
import concourse.mybir as mybir  # noqa: E402
import concourse.tile as tile  # noqa: E402
from concourse import bacc  # noqa: E402
from concourse.bass_utils import run_bass_kernel_spmd  # noqa: E402

P = 128
T, D, I, E, TOPK = 4096, 1024, 1024, 8, 2
NCORES = 8
DK = D // P          # 8
IK = I // P          # 8
NQ = 4               # ReduceScatter token ranges
RT = T // NQ         # 1024 tokens per range
RSH = RT // NCORES   # 128-token shard per core per range
CAP = 288            # routed-token capacity per (core, range)
CT = 3               # c-tiles per range (128, 128, 32 rows)
TPP = 512            # tokens per gate piece (= per-core gate shard)
XPAD_ROWS = T + P    # x padded with zero rows (pad-slot gather target)
# chunk order for AG-derived ranges: col j of a range maps to token base
# CHORD[j]*P within the range (chunk index c = 4*j + tt)
CHORD = [0, 4, 1, 5, 2, 6, 3, 7]
f32 = mybir.dt.float32
bf16 = mybir.dt.bfloat16
f16 = mybir.dt.float16
i32 = mybir.dt.int32
BF = mybir.dt.np(bf16)

_CACHED_NC = None


def _ct_rows(ct):
    return min(P, CAP - ct * P)


def _build():
    nc = bacc.Bacc("TRN2", target_bir_lowering=False, debug=False,
                   num_devices=NCORES)
    # xgT slots: [piece0, piece1, own piece] each [P, DK*TPP] pre-shuffled
    xgT_d = nc.dram_tensor("xgT", [P, 3 * DK * TPP], f32,
                           kind="ExternalInput")
    x_d = nc.dram_tensor("x", [XPAD_ROWS, D], bf16, kind="ExternalInput")
    gwTn_d = nc.dram_tensor("gwTn", [P, DK * E], f32, kind="ExternalInput")
    gwTp_d = nc.dram_tensor("gwTp", [P, DK * E], f32, kind="ExternalInput")
    w1T_d = nc.dram_tensor("w1T", [P, DK * I], bf16, kind="ExternalInput")
    w3T_d = nc.dram_tensor("w3T", [P, DK * I], bf16, kind="ExternalInput")
    w2T_d = nc.dram_tensor("w2T", [P, IK * D], bf16, kind="ExternalInput")
    utri_d = nc.dram_tensor("utri", [P, P], f16, kind="ExternalInput")
    ones_d = nc.dram_tensor("ones", [P, P], f16, kind="ExternalInput")
    identf_d = nc.dram_tensor("identf", [P, P], f32, kind="ExternalInput")
    identb_d = nc.dram_tensor("identb", [P, P], bf16, kind="ExternalInput")
    tidb_d = nc.dram_tensor("tidb", [P, 2 * E], f32, kind="ExternalInput")
    sr_d = nc.dram_tensor("sr", [P, CT * P], f32, kind="ExternalInput")
    selidx_d = nc.dram_tensor("selidx", [E, 1], i32, kind="ExternalInput")
    y_d = nc.dram_tensor("y", [NQ * RSH, D], bf16, kind="ExternalOutput")

    with tile.TileContext(nc) as tc:
        with tc.tile_pool(name="wpool", bufs=1) as wpool, \
             tc.tile_pool(name="xgpool", bufs=3) as xgpool, \
             tc.tile_pool(name="gpool", bufs=2) as gpool, \
             tc.tile_pool(name="wgpool", bufs=1) as wgpool, \
             tc.tile_pool(name="cpool", bufs=3) as cpool, \
             tc.tile_pool(name="xepool", bufs=3) as xepool, \
             tc.tile_pool(name="xtpool", bufs=2) as xtpool, \
             tc.tile_pool(name="apool", bufs=2) as apool, \
             tc.tile_pool(name="spool", bufs=2) as spool, \
             tc.tile_pool(name="ypool", bufs=2) as ypool, \
             tc.tile_pool(name="psum", bufs=2, space="PSUM") as psum, \
             tc.tile_pool(name="pyps", bufs=2, space="PSUM") as pyps, \
             tc.tile_pool(name="psmall", bufs=2, space="PSUM") as psmall, \
             tc.tile_pool(name="dram", bufs=1, space="DRAM") as dram:

            # --- dummy 4B AllGather: absorbs the one-time CC bootstrap +
            #     launch skew while weights/gate are still loading ---
            ccz = wpool.tile([1, 1], f32, tag="ccz")
            nc.vector.memset(ccz[:], 0.0)
            ccin_d = dram.tile([1, 1], f32, tag="cci", name="cci")
            nc.gpsimd.dma_start(ccin_d[:, :], ccz[:])
            ccwarm_d = dram.tile([NCORES, 1], f32, tag="ccw", name="ccw")
            nc.gpsimd.collective_compute(
                "AllGather",
                mybir.AluOpType.bypass,
                replica_groups=[list(range(NCORES))],
                ins=[ccin_d[:, :].opt()],
                outs=[ccwarm_d[:, :].opt()],
            )

            # --- constants + gate inputs (sync queue) ---
            gwTn_s = wpool.tile([P, DK, E], f32, tag="gwn")
            nc.sync.dma_start(
                gwTn_s[:], gwTn_d[:, :].rearrange("p (o e) -> p o e", e=E))
            gwTp_s = wpool.tile([P, DK, E], f32, tag="gwp")
            nc.sync.dma_start(
                gwTp_s[:], gwTp_d[:, :].rearrange("p (o e) -> p o e", e=E))
            utri_s = wpool.tile([P, P], f16, tag="utri")
            nc.sync.dma_start(utri_s[:], utri_d[:, :])
            ones_s = wpool.tile([P, P], f16, tag="ones")
            nc.sync.dma_start(ones_s[:], ones_d[:, :])
            identf_s = wpool.tile([P, P], f32, tag="identf")
            nc.sync.dma_start(identf_s[:], identf_d[:, :])
            identb_s = wpool.tile([P, P], bf16, tag="identb")
            nc.sync.dma_start(identb_s[:], identb_d[:, :])
            tidb_s = wpool.tile([P, 2, E], f32, tag="tidb")
            nc.sync.dma_start(
                tidb_s[:], tidb_d[:, :].rearrange("p (s e) -> p s e", e=E))
            sr_s = wpool.tile([P, CT * P], f32, tag="sr")
            nc.sync.dma_start(sr_s[:], sr_d[:, :])
            selidx_s = wpool.tile([E, 1], i32, tag="selidx")
            nc.sync.dma_start(selidx_s[:], selidx_d[:, :])
            xgp = []
            for s in range(3):
                t = xgpool.tile([P, DK, TPP], f32, tag="xgp", name=f"xgp{s}")
                nc.sync.dma_start(
                    t[:],
                    xgT_d[:, s * DK * TPP:(s + 1) * DK * TPP].rearrange(
                        "p (o t) -> p o t", t=TPP))
                xgp.append(t)

            # --- expert weights (scalar + gpsimd queues) ---
            w1T_s = wpool.tile([P, DK, I], bf16, tag="w1")
            w3T_s = wpool.tile([P, DK, I], bf16, tag="w3")
            w2T_s = wpool.tile([P, IK, D], bf16, tag="w2")
            for h in range(2):
                osl = slice(h * (DK // 2), (h + 1) * (DK // 2))
                fsl = slice(h * (DK // 2) * I, (h + 1) * (DK // 2) * I)
                nc.scalar.dma_start(
                    w1T_s[:, osl, :],
                    w1T_d[:, fsl].rearrange("p (o i) -> p o i", i=I))
                nc.gpsimd.dma_start(
                    w3T_s[:, osl, :],
                    w3T_d[:, fsl].rearrange("p (o i) -> p o i", i=I))
                nc.scalar.dma_start(
                    w2T_s[:, osl, :],
                    w2T_d[:, fsl].rearrange("p (o i) -> p o i", i=D))

            # --- contribution buffers, zero-filled early ---
            ycontribs = [dram.tile([RT, D], bf16, tag=f"yc{q}",
                                   name=f"yc{q}") for q in range(NQ)]
            yshards = [dram.tile([RSH, D], bf16, tag=f"ys{q}", name=f"ys{q}")
                       for q in range(NQ)]
            # zero pattern: mapping is irrelevant, so use the partition-
            # contiguous rearrange (16KB per descriptor)
            zt = wpool.tile([P, RT // P, D], bf16, tag="zt")
            nc.vector.memset(zt[:], 0.0)
            for q in range(NQ):
                eng = nc.sync if q % 2 == 0 else nc.scalar
                eng.dma_start(
                    ycontribs[q][:, :].rearrange("(p j) d -> p j d", p=P),
                    zt[:])

            def softmax_top2(ps_g):
                """probs [P, E] (softmax) and mx8 [P, 8] from scores psum."""
                negmx = gpool.tile([P, 1], f32, tag="negmx")
                nc.vector.tensor_reduce(
                    negmx[:], ps_g[:], mybir.AxisListType.X,
                    mybir.AluOpType.max)
                nc.vector.tensor_scalar_mul(negmx[:], negmx[:], -1.0)
                probs = gpool.tile([P, E], f32, tag="probs")
                sumexp = gpool.tile([P, 1], f32, tag="sumexp")
                nc.scalar.activation(
                    probs[:], ps_g[:], mybir.ActivationFunctionType.Exp,
                    bias=negmx[:, 0:1], accum_out=sumexp[:, 0:1])
                recip = gpool.tile([P, 1], f32, tag="recip")
                nc.vector.reciprocal(recip[:], sumexp[:])
                nc.vector.tensor_scalar_mul(probs[:], probs[:], recip[:, 0:1])
                mx8 = gpool.tile([P, 8], f32, tag="mx8")
                nc.vector.max(mx8[:], probs[:])
                return probs, mx8

            # ---- local fp32 gate for range 0 (permuted: own expert col 0)
            wgtq0 = wgpool.tile([P, E], f32, tag="wgtq0")
            for g in range(2):
                ps_sT = psmall.tile([E, TPP], f32, tag="sm")
                for dk in range(DK):
                    nc.tensor.matmul(
                        ps_sT[:], lhsT=gwTp_s[:, dk, :], rhs=xgp[g][:, dk, :],
                        start=(dk == 0), stop=(dk == DK - 1))
                sT_sb = gpool.tile([E, TPP], f32, tag="sTsb")
                nc.vector.tensor_copy(sT_sb[:], ps_sT[:])
                for tt in range(TPP // P):
                    f = g * (TPP // P) + tt
                    ps_g = psmall.tile([P, E], f32, tag="sm")
                    nc.tensor.transpose(
                        ps_g[:], sT_sb[:, tt * P:(tt + 1) * P],
                        identf_s[:E, :E])
                    probs, mx8 = softmax_top2(ps_g)
                    ge = gpool.tile([P, 1], f32, tag="ge")
                    nc.vector.tensor_tensor(
                        ge[:], probs[:, 0:1], mx8[:, 1:2],
                        mybir.AluOpType.is_ge)
                    nc.vector.tensor_mul(
                        wgtq0[:, f:f + 1], probs[:, 0:1], ge[:])

            # ---- sharded fp32 gate on own 512 tokens (natural order) ----
            ps_sT = psmall.tile([E, TPP], f32, tag="sm")
            for dk in range(DK):
                nc.tensor.matmul(
                    ps_sT[:], lhsT=gwTn_s[:, dk, :], rhs=xgp[2][:, dk, :],
                    start=(dk == 0), stop=(dk == DK - 1))
            sT_sb = gpool.tile([E, TPP], f32, tag="sTsb")
            nc.vector.tensor_copy(sT_sb[:], ps_sT[:])
            wg_loc = wgpool.tile([P, TPP // P, E], f32, tag="wgloc")
            for tt in range(TPP // P):
                ps_g = psmall.tile([P, E], f32, tag="sm")
                nc.tensor.transpose(
                    ps_g[:], sT_sb[:, tt * P:(tt + 1) * P], identf_s[:E, :E])
                probs, mx8 = softmax_top2(ps_g)
                ge8 = gpool.tile([P, E], f32, tag="ge8")
                nc.vector.tensor_tensor(
                    ge8[:], probs[:], mx8[:, 1:2].to_broadcast([P, E]),
                    mybir.AluOpType.is_ge)
                nc.vector.tensor_mul(wg_loc[:, tt, :], probs[:], ge8[:])
            # back to expert-major [E, TPP] and publish
            wgT_loc = wgpool.tile([E, TPP], f32, tag="wgTloc")
            for tt in range(TPP // P):
                psb = psmall.tile([E, P], f32, tag="sm")
                nc.tensor.transpose(psb[:], wg_loc[:, tt, :], identf_s[:])
                nc.vector.tensor_copy(wgT_loc[:, tt * P:(tt + 1) * P], psb[:])
            wgl_d = dram.tile([E, TPP], f32, tag="wgl", name="wgl")
            nc.sync.dma_start(wgl_d[:, :], wgT_loc[:])

            wgallT_d = dram.tile([NCORES * E, TPP], f32, tag="wgall",
                                 name="wgall", addr_space="Shared")
            nc.gpsimd.collective_compute(
                "AllGather",
                mybir.AluOpType.bypass,
                replica_groups=[list(range(NCORES))],
                ins=[wgl_d[:, :].opt()],
                outs=[wgallT_d[:, :].opt()],
            )
            # wgAT extraction is DEFERRED into the loop (after range 0's
            # scatter + RS trigger): emitted any earlier, its gpsimd
            # gather and PE transposes sit ahead of range-0 work on the
            # in-order queues and stall phase C on the AllGather.
            wgAT_s = wgpool.tile([E, TPP], f32, tag="wgAT")
            wgt32v = wgpool.tile([P, TPP // P, E], f32, tag="wgt32")

            def load_wgAT():
                # rows (r'*E + my_e) of the allgathered [64, TPP], then
                # token-partitions: wgt32v[:, tt, j] = weight of token
                # j*TPP + tt*P + p  (chunk index c = 4*j + tt)
                nc.gpsimd.indirect_dma_start(
                    out=wgAT_s[:],
                    out_offset=None,
                    in_=wgallT_d[:, :],
                    in_offset=bass.IndirectOffsetOnAxis(
                        ap=selidx_s[:, 0:1], axis=0))
                for tt in range(TPP // P):
                    pst = psmall.tile([P, E], f32, tag="sm")
                    nc.tensor.transpose(
                        pst[:], wgAT_s[:, tt * P:(tt + 1) * P],
                        identf_s[:E, :E])
                    nc.vector.tensor_copy(wgt32v[:, tt, :], pst[:])

            # ===== phase B + C, software-pipelined over ranges =====
            def compact(q):
                """Prefix-sum slot assignment + fp16 one-hot list build,
                all on PE/DVE.  Returns (lst, gidx_i, yidx_i)."""
                if q == 0:
                    wq = wgtq0[:]        # natural chunk order
                    tid = tidb_s[:, 0, :]
                else:
                    wq = wgt32v[:, :, 2 * q:2 * q + 2]   # CHORD order
                    tid = tidb_s[:, 1, :]
                m = cpool.tile([P, E], f16, tag="m", name=f"m{q}")
                nc.vector.tensor_scalar(
                    m[:], wq, 0.0, scalar2=None, op0=mybir.AluOpType.is_gt)
                psA = psmall.tile([P, E], f32, tag="sm")
                nc.tensor.matmul(psA[:], lhsT=utri_s[:], rhs=m[:],
                                 start=True, stop=True)
                psC = psmall.tile([P, E], f32, tag="sm")
                nc.tensor.matmul(psC[:], lhsT=ones_s[:], rhs=m[:],
                                 start=True, stop=True)
                pos = cpool.tile([P, E], f32, tag="pos", name=f"pos{q}")
                nc.vector.tensor_copy(pos[:], psA[:])
                ctot = cpool.tile([P, E], f32, tag="ctot", name=f"ct{q}")
                nc.vector.tensor_copy(ctot[:], psC[:])
                for f in range(1, E):
                    nc.vector.tensor_add(
                        ctot[:, f:f + 1], ctot[:, f:f + 1], ctot[:, f - 1:f])
                for f in range(1, E):
                    nc.vector.tensor_add(
                        pos[:, f:f + 1], pos[:, f:f + 1], ctot[:, f - 1:f])
                # non-routed tokens park at pos = RT (matches no slot)
                nc.vector.tensor_scalar_add(pos[:], pos[:], float(-RT))
                nc.vector.tensor_mul(pos[:], pos[:], m[:])
                nc.vector.tensor_scalar_add(pos[:], pos[:], float(RT))

                # fp16 payload: [tid_local (<=1023, exact), wgt, occ]
                pay = cpool.tile([P, E, 3], f16, tag="pay", name=f"pay{q}")
                nc.vector.tensor_copy(pay[:, :, 0], tid)
                nc.vector.tensor_copy(pay[:, :, 1], wq)
                nc.vector.tensor_copy(pay[:, :, 2], m[:])

                lst = cpool.tile([P, CT, 3], f32, tag="lst", name=f"lst{q}")
                for ct in range(CT):
                    ps_l = psmall.tile([P, 3], f32, tag="sm")
                    for f in range(E):
                        ind = cpool.tile([P, P], f16, tag="ind")
                        nc.vector.tensor_tensor(
                            ind[:], pos[:, f:f + 1].to_broadcast([P, P]),
                            sr_s[:, ct * P:(ct + 1) * P],
                            mybir.AluOpType.is_equal)
                        nc.tensor.matmul(
                            ps_l[:], lhsT=ind[:], rhs=pay[:, f, :],
                            start=(f == 0), stop=(f == E - 1))
                    nc.vector.tensor_copy(lst[:, ct, :], ps_l[:])

                # pads (occ=0): gather the zero x row; scatter is dropped
                # by the bounds check (yidx = RT + tid_local >= RT)
                occ1 = cpool.tile([P, CT], f32, tag="occ1", name=f"occ1{q}")
                nc.vector.tensor_scalar(
                    occ1[:], lst[:, :, 2], -1.0, None,
                    op0=mybir.AluOpType.add)        # occ-1  (0 or -1)
                gidxf = cpool.tile([P, CT], f32, tag="gxf", name=f"gxf{q}")
                nc.vector.tensor_scalar(
                    gidxf[:], occ1[:], -float(T - q * RT), None,
                    op0=mybir.AluOpType.mult)       # (1-occ)*(T - q*RT)
                nc.vector.tensor_add(gidxf[:], gidxf[:], lst[:, :, 0])
                nc.vector.tensor_scalar_add(
                    gidxf[:], gidxf[:], float(q * RT))
                gidx_i = cpool.tile([P, CT], i32, tag="gidx", name=f"gi{q}")
                nc.vector.tensor_copy(gidx_i[:], gidxf[:])
                yidxf = cpool.tile([P, CT], f32, tag="yxf", name=f"yxf{q}")
                nc.vector.tensor_scalar(
                    yidxf[:], occ1[:], -float(RT), None,
                    op0=mybir.AluOpType.mult)       # (1-occ)*RT
                nc.vector.tensor_add(yidxf[:], yidxf[:], lst[:, :, 0])
                yidx_i = cpool.tile([P, CT], i32, tag="yidxi", name=f"yi{q}")
                nc.vector.tensor_copy(yidx_i[:], yidxf[:])
                return lst, gidx_i, yidx_i

            def gather_transpose(q, gidx_i):
                """Per c-tile indirect x-row gathers + PE transposes."""
                xeT = xtpool.tile([P, DK, CAP], bf16, tag="xeT",
                                  name=f"xeT{q}")
                for ct in range(CT):
                    rows = _ct_rows(ct)
                    xe = xepool.tile([P, D], bf16, tag="xe")
                    nc.gpsimd.indirect_dma_start(
                        out=xe[0:rows, :],
                        out_offset=None,
                        in_=x_d[:, :],
                        in_offset=bass.IndirectOffsetOnAxis(
                            ap=gidx_i[0:rows, ct:ct + 1], axis=0))
                    for dk in range(DK):
                        ptr = psmall.tile([P, P], bf16, tag="sm")
                        nc.tensor.transpose(
                            ptr[0:P, 0:rows],
                            xe[0:rows, dk * P:(dk + 1) * P],
                            identb_s[0:rows, 0:rows])
                        nc.scalar.copy(
                            xeT[:, dk, ct * P:ct * P + rows], ptr[:, 0:rows])
                return xeT

            state = {0: compact(0)}
            xeT_cur = gather_transpose(0, state[0][1])

            for q in range(NQ):
                lst, gidx_i, yidx_i = state.pop(q)

                aT = apool.tile([P, IK, CAP], bf16, tag="aT")
                for ik in range(IK):
                    isl = slice(ik * P, (ik + 1) * P)
                    ph = psum.tile([P, CAP], f32, tag="ph")
                    for dk in range(DK):
                        nc.tensor.matmul(
                            ph[:], lhsT=w1T_s[:, dk, isl],
                            rhs=xeT_cur[:, dk, :],
                            start=(dk == 0), stop=(dk == DK - 1))
                    pg = psum.tile([P, CAP], f32, tag="pg")
                    for dk in range(DK):
                        nc.tensor.matmul(
                            pg[:], lhsT=w3T_s[:, dk, isl],
                            rhs=xeT_cur[:, dk, :],
                            start=(dk == 0), stop=(dk == DK - 1))
                    sil = spool.tile([P, CAP], bf16, tag="sil")
                    nc.scalar.activation(
                        sil[:], ph[:], mybir.ActivationFunctionType.Silu)
                    nc.vector.tensor_mul(aT[:, ik, :], sil[:], pg[:])



                for ct in range(CT):
                    rows = _ct_rows(ct)
                    yt = ypool.tile([P, D], bf16, tag="yt")
                    for dc in range(2):
                        py = pyps.tile([P, D // 2], f32, tag="py")
                        for ik in range(IK):
                            nc.tensor.matmul(
                                py[0:rows, :],
                                lhsT=aT[:, ik, ct * P:ct * P + rows],
                                rhs=w2T_s[:, ik,
                                          dc * (D // 2):(dc + 1) * (D // 2)],
                                start=(ik == 0), stop=(ik == IK - 1))
                        nc.vector.tensor_scalar_mul(
                            yt[0:rows, dc * (D // 2):(dc + 1) * (D // 2)],
                            py[0:rows, :], lst[0:rows, ct, 1:2])
                    # pad rows (yidx >= RT) are dropped by the bounds check
                    nc.gpsimd.indirect_dma_start(
                        out=ycontribs[q][:, :],
                        out_offset=bass.IndirectOffsetOnAxis(
                            ap=yidx_i[0:rows, ct:ct + 1], axis=0),
                        in_=yt[0:rows, :],
                        in_offset=None,
                        bounds_check=RT - 1,
                        oob_is_err=False)

                nc.gpsimd.collective_compute(
                    "ReduceScatter",
                    mybir.AluOpType.add,
                    replica_groups=[list(range(NCORES))],
                    ins=[ycontribs[q][:, :].opt()],
                    outs=[yshards[q].opt()],
                )

                # next range's AG-dependent prep goes AFTER this range's
                # scatter + RS trigger: RS(q) can then never be delayed by
                # the AllGather, even on a slow-barrier run
                if q == 0:
                    load_wgAT()
                if q + 1 < NQ:
                    state[q + 1] = compact(q + 1)
                    xeT_cur = gather_transpose(q + 1, state[q + 1][1])

            # ship shards at the very end: a mid-pipeline y write would
            # make later ranges' work wait on the collective
            for q in range(NQ):
                nc.sync.dma_start(y_d[q * RSH:(q + 1) * RSH, :], yshards[q][:])
    nc.compile()
    return nc


def _get_nc():
    global _CACHED_NC
    if _CACHED_NC is None:
        _CACHED_NC = _build()
    return _CACHED_NC


def _preshuffle(mat, nk):
    """[nk*P, M] -> [P, nk*M] with row p = concat_o mat[o*P + p, :]."""
    nkP, M = mat.shape
    assert nkP == nk * P
    return np.ascontiguousarray(
        mat.reshape(nk, P, M).transpose(1, 0, 2).reshape(P, nk * M))


def _in_maps(x, gate_w, w1, w3, w2):
    x = np.asarray(x, dtype=np.float32)
    gate_w = np.asarray(gate_w, dtype=np.float32)
    xpad = np.zeros((XPAD_ROWS, D), dtype=BF)
    xpad[:T] = x.astype(BF)

    # host-side capacity check against the actual gate (cheap, exact)
    s = x @ gate_w.T
    thr = np.sort(s, axis=1)[:, -TOPK]          # 2nd-largest score
    routed = s >= thr[:, None]                  # [T, E]
    cnt = routed.reshape(NQ, RT, E).sum(axis=1)  # [NQ, E]
    if cnt.max() > CAP:
        raise RuntimeError(f"routing capacity exceeded: {cnt.max()} > {CAP}")

    F16 = np.float16
    utri = np.triu(np.ones((P, P), np.float32), k=1).astype(F16)
    ones = np.ones((P, P), F16)
    identf = np.eye(P, dtype=np.float32)
    identb = np.eye(P, dtype=np.float32).astype(BF)
    # tidb holds LOCAL (within-range) token bases, exact in fp16
    # slot 0: natural order (range 0); slot 1: CHORD order (AG ranges)
    tidb_nat = (np.arange(E, dtype=np.float32)[None, :] * P
                + np.arange(P, dtype=np.float32)[:, None])
    tidb_ch = (np.array(CHORD, np.float32)[None, :] * P
               + np.arange(P, dtype=np.float32)[:, None])
    tidb = np.concatenate([tidb_nat, tidb_ch], axis=1).astype(np.float32)
    sr = np.broadcast_to(np.arange(CT * P, dtype=np.float32)[None, :],
                         (P, CT * P)).copy()
    gwTn_pre = _preshuffle(np.ascontiguousarray(gate_w.T), DK)
    # per-512-token gate pieces, pre-shuffled [P, DK*TPP] each
    xT = np.ascontiguousarray(x.T)  # [D, T]
    pieces = [
        np.ascontiguousarray(
            xT[:, g * TPP:(g + 1) * TPP].reshape(DK, P, TPP)
            .transpose(1, 0, 2).reshape(P, DK * TPP))
        for g in range(NCORES)
    ]

    maps = []
    for e in range(NCORES):
        perm = [e] + [j for j in range(E) if j != e]
        gwTp_pre = _preshuffle(np.ascontiguousarray(gate_w[perm].T), DK)
        maps.append({
            "xgT": np.concatenate([pieces[0], pieces[1], pieces[e]], axis=1),
            "x": xpad,
            "gwTn": gwTn_pre,
            "gwTp": gwTp_pre,
            "w1T": _preshuffle(
                np.ascontiguousarray(np.asarray(w1[e], np.float32).T), DK
            ).astype(BF),
            "w3T": _preshuffle(
                np.ascontiguousarray(np.asarray(w3[e], np.float32).T), DK
            ).astype(BF),
            "w2T": _preshuffle(
                np.ascontiguousarray(np.asarray(w2[e], np.float32).T), IK
            ).astype(BF),
            "utri": utri,
            "ones": ones,
            "identf": identf,
            "identb": identb,
            "tidb": tidb,
            "sr": sr,
            "selidx": (np.arange(NCORES, dtype=np.int32)[:, None] * E
                       + np.int32(e)),
        })
    return maps


def run(x, gate_w, w1, w3, w2, trace=False, trace_cores=None):
    nc = _get_nc()
    maps = _in_maps(x, gate_w, w1, w3, w2)
    res = run_bass_kernel_spmd(
        nc, maps, core_ids=list(range(NCORES)), trace=trace,
        trace_cores=trace_cores)
    # core r's output block q (128 rows) holds tokens [1024q + 128r, +128)
    y = np.empty((T, D), dtype=np.float32)
    for r in range(NCORES):
        yr = np.asarray(res.results[r]["y"]).astype(np.float32)
        for q in range(NQ):
            t0 = q * RT + r * RSH
            y[t0:t0 + RSH] = yr[q * RSH:(q + 1) * RSH]
    return y, res


def kernel(x, gate_w, w1, w3, w2):
    y, _ = run(x, gate_w, w1, w3, w2, trace=False)
    return y.astype(np.float32)


# revision 33
# speedup vs baseline: 1.1283x; 1.1283x over previous
"""MoE SwiGLU (T=4096, D=I=1024, E=8, top-2) on 8 Trainium2 NeuronCores.

Expert-parallel with on-device routing, v7:

- Gate, hybrid: every core computes the fp32 gate for ITS 512 tokens
  (scores-major, natural expert order), publishes all-expert weights
  [E, 512] and a 16KB AllGather replicates them; each core then pulls its
  own expert's rows via an indirect row gather (per-core index input).
  Range 0 (tokens 0..1023) is ALSO gated locally with permuted gate
  weights (own expert = column 0), so phase C starts without waiting for
  the collective; a 4-byte dummy AllGather issued at t=0 absorbs the
  one-time CC bootstrap + launch skew while weights load.
- bf16 on the whole expert path; fp32 PSUM accumulate.
- Compaction entirely on PE+DVE (no DMA round trip, no queue hazards):
  matmul prefix-sums assign slots, then per 128-slot tile an fp16 one-hot
  (pos == slot) matmul accumulates each token's (tid_local, wgt, occ)
  payload into the compact list (fp16 keeps tid_local <= 1023 exact and
  its LDWEIGHTS at full rate).
- Phase C per range (4 x 1024 tokens): per c-tile indirect gather of
  routed x rows (pads redirected to the zero row), PE transposes to
  [D, tokens], SwiGLU, routing-weight scale, per c-tile indirect scatter
  into a zeroed [RT, D] bf16 contribution buffer (pad rows dropped by
  the DGE bounds check), bf16 ReduceScatter per range; shards ship to y
  at the very end (no mid-pipeline collective waits on any queue).
  Ranges are software-pipelined: compaction of q+1 goes before the
  w1/w3 stage of q, gathers+transposes of q+1 between w1/w3 and w2.
- Host-side pre-shuffled DRAM layouts keep big DMAs at >=4KB/descriptor.

Capacity: per (core, range) routed-token count for the fixed test seed is
256 +- 25 (max 281); CAP=288 with a host-side overflow check.
"""
import os
import sys

import numpy as np

for _p in ("/opt/trn_rl_repo", "/root/.axon_site/_ro/trn_rl_repo"):
    if os.path.isdir(_p) and _p not in sys.path:
        sys.path.append(_p)

import concourse.bass as bass  # noqa: E402
import concourse.mybir as mybir  # noqa: E402
import concourse.tile as tile  # noqa: E402
from concourse import bacc  # noqa: E402
from concourse.bass_utils import run_bass_kernel_spmd  # noqa: E402

P = 128
T, D, I, E, TOPK = 4096, 1024, 1024, 8, 2
NCORES = 8
DK = D // P          # 8
IK = I // P          # 8
NQ = 4               # ReduceScatter token ranges
RT = T // NQ         # 1024 tokens per range
RSH = RT // NCORES   # 128-token shard per core per range
CAP = 288            # routed-token capacity per (core, range)
CT = 3               # c-tiles per range (128, 128, 32 rows)
TPP = 512            # tokens per gate piece (= per-core gate shard)
XPAD_ROWS = T + P    # x padded with zero rows (pad-slot gather target)
# chunk order for AG-derived ranges: col j of a range maps to token base
# CHORD[j]*P within the range (chunk index c = 4*j + tt)
CHORD = [0, 4, 1, 5, 2, 6, 3, 7]
f32 = mybir.dt.float32
bf16 = mybir.dt.bfloat16
f16 = mybir.dt.float16
i32 = mybir.dt.int32
BF = mybir.dt.np(bf16)

_CACHED_NC = None


def _ct_rows(ct):
    return min(P, CAP - ct * P)


def _build():
    nc = bacc.Bacc("TRN2", target_bir_lowering=False, debug=False,
                   num_devices=NCORES)
    # xgT slots: [piece0, piece1, own piece] each [P, DK*TPP] pre-shuffled
    xgT_d = nc.dram_tensor("xgT", [P, 3 * DK * TPP], f32,
                           kind="ExternalInput")
    x_d = nc.dram_tensor("x", [XPAD_ROWS, D], bf16, kind="ExternalInput")
    gwTn_d = nc.dram_tensor("gwTn", [P, DK * E], f32, kind="ExternalInput")
    gwTp_d = nc.dram_tensor("gwTp", [P, DK * E], f32, kind="ExternalInput")
    w1T_d = nc.dram_tensor("w1T", [P, DK * I], bf16, kind="ExternalInput")
    w3T_d = nc.dram_tensor("w3T", [P, DK * I], bf16, kind="ExternalInput")
    w2T_d = nc.dram_tensor("w2T", [P, IK * D], bf16, kind="ExternalInput")
    utri_d = nc.dram_tensor("utri", [P, P], f16, kind="ExternalInput")
    ones_d = nc.dram_tensor("ones", [P, P], f16, kind="ExternalInput")
    identf_d = nc.dram_tensor("identf", [P, P], f32, kind="ExternalInput")
    identb_d = nc.dram_tensor("identb", [P, P], bf16, kind="ExternalInput")
    tidb_d = nc.dram_tensor("tidb", [P, 2 * E], f32, kind="ExternalInput")
    sr_d = nc.dram_tensor("sr", [P, CT * P], f32, kind="ExternalInput")
    selidx_d = nc.dram_tensor("selidx", [E, 1], i32, kind="ExternalInput")
    y_d = nc.dram_tensor("y", [NQ * RSH, D], bf16, kind="ExternalOutput")

    with tile.TileContext(nc) as tc:
        with tc.tile_pool(name="wpool", bufs=1) as wpool, \
             tc.tile_pool(name="xgpool", bufs=3) as xgpool, \
             tc.tile_pool(name="gpool", bufs=2) as gpool, \
             tc.tile_pool(name="wgpool", bufs=1) as wgpool, \
             tc.tile_pool(name="cpool", bufs=3) as cpool, \
             tc.tile_pool(name="xepool", bufs=3) as xepool, \
             tc.tile_pool(name="xtpool", bufs=2) as xtpool, \
             tc.tile_pool(name="apool", bufs=2) as apool, \
             tc.tile_pool(name="spool", bufs=2) as spool, \
             tc.tile_pool(name="ypool", bufs=2) as ypool, \
             tc.tile_pool(name="psum", bufs=2, space="PSUM") as psum, \
             tc.tile_pool(name="pyps", bufs=2, space="PSUM") as pyps, \
             tc.tile_pool(name="psmall", bufs=2, space="PSUM") as psmall, \
             tc.tile_pool(name="dram", bufs=1, space="DRAM") as dram:

            # --- dummy 4B AllGather: absorbs the one-time CC bootstrap +
            #     launch skew while weights/gate are still loading ---
            ccz = wpool.tile([1, 1], f32, tag="ccz")
            nc.vector.memset(ccz[:], 0.0)
            ccin_d = dram.tile([1, 1], f32, tag="cci", name="cci")
            nc.gpsimd.dma_start(ccin_d[:, :], ccz[:])
            ccwarm_d = dram.tile([NCORES, 1], f32, tag="ccw", name="ccw")
            nc.gpsimd.collective_compute(
                "AllGather",
                mybir.AluOpType.bypass,
                replica_groups=[list(range(NCORES))],
                ins=[ccin_d[:, :].opt()],
                outs=[ccwarm_d[:, :].opt()],
            )

            # --- constants + gate inputs (sync queue) ---
            gwTn_s = wpool.tile([P, DK, E], f32, tag="gwn")
            nc.sync.dma_start(
                gwTn_s[:], gwTn_d[:, :].rearrange("p (o e) -> p o e", e=E))
            gwTp_s = wpool.tile([P, DK, E], f32, tag="gwp")
            nc.sync.dma_start(
                gwTp_s[:], gwTp_d[:, :].rearrange("p (o e) -> p o e", e=E))
            utri_s = wpool.tile([P, P], f16, tag="utri")
            nc.sync.dma_start(utri_s[:], utri_d[:, :])
            ones_s = wpool.tile([P, P], f16, tag="ones")
            nc.sync.dma_start(ones_s[:], ones_d[:, :])
            identf_s = wpool.tile([P, P], f32, tag="identf")
            nc.sync.dma_start(identf_s[:], identf_d[:, :])
            identb_s = wpool.tile([P, P], bf16, tag="identb")
            nc.sync.dma_start(identb_s[:], identb_d[:, :])
            tidb_s = wpool.tile([P, 2, E], f32, tag="tidb")
            nc.sync.dma_start(
                tidb_s[:], tidb_d[:, :].rearrange("p (s e) -> p s e", e=E))
            sr_s = wpool.tile([P, CT * P], f32, tag="sr")
            nc.sync.dma_start(sr_s[:], sr_d[:, :])
            selidx_s = wpool.tile([E, 1], i32, tag="selidx")
            nc.sync.dma_start(selidx_s[:], selidx_d[:, :])
            xgp = []
            for s in range(3):
                t = xgpool.tile([P, DK, TPP], f32, tag="xgp", name=f"xgp{s}")
                nc.sync.dma_start(
                    t[:],
                    xgT_d[:, s * DK * TPP:(s + 1) * DK * TPP].rearrange(
                        "p (o t) -> p o t", t=TPP))
                xgp.append(t)

            # --- expert weights (scalar + gpsimd queues) ---
            w1T_s = wpool.tile([P, DK, I], bf16, tag="w1")
            w3T_s = wpool.tile([P, DK, I], bf16, tag="w3")
            w2T_s = wpool.tile([P, IK, D], bf16, tag="w2")
            for h in range(2):
                osl = slice(h * (DK // 2), (h + 1) * (DK // 2))
                fsl = slice(h * (DK // 2) * I, (h + 1) * (DK // 2) * I)
                nc.scalar.dma_start(
                    w1T_s[:, osl, :],
                    w1T_d[:, fsl].rearrange("p (o i) -> p o i", i=I))
                nc.gpsimd.dma_start(
                    w3T_s[:, osl, :],
                    w3T_d[:, fsl].rearrange("p (o i) -> p o i", i=I))
                nc.scalar.dma_start(
                    w2T_s[:, osl, :],
                    w2T_d[:, fsl].rearrange("p (o i) -> p o i", i=D))

            # --- contribution buffers, zero-filled early ---
            ycontribs = [dram.tile([RT, D], bf16, tag=f"yc{q}",
                                   name=f"yc{q}") for q in range(NQ)]
            yshards = [dram.tile([RSH, D], bf16, tag=f"ys{q}", name=f"ys{q}")
                       for q in range(NQ)]
            # zero pattern: mapping is irrelevant, so use the partition-
            # contiguous rearrange (16KB per descriptor)
            zt = wpool.tile([P, RT // P, D], bf16, tag="zt")
            nc.vector.memset(zt[:], 0.0)
            for q in range(NQ):
                eng = nc.sync if q % 2 == 0 else nc.scalar
                eng.dma_start(
                    ycontribs[q][:, :].rearrange("(p j) d -> p j d", p=P),
                    zt[:])

            def softmax_top2(ps_g):
                """probs [P, E] (softmax) and mx8 [P, 8] from scores psum."""
                negmx = gpool.tile([P, 1], f32, tag="negmx")
                nc.vector.tensor_reduce(
                    negmx[:], ps_g[:], mybir.AxisListType.X,
                    mybir.AluOpType.max)
                nc.vector.tensor_scalar_mul(negmx[:], negmx[:], -1.0)
                probs = gpool.tile([P, E], f32, tag="probs")
                sumexp = gpool.tile([P, 1], f32, tag="sumexp")
                nc.scalar.activation(
                    probs[:], ps_g[:], mybir.ActivationFunctionType.Exp,
                    bias=negmx[:, 0:1], accum_out=sumexp[:, 0:1])
                recip = gpool.tile([P, 1], f32, tag="recip")
                nc.vector.reciprocal(recip[:], sumexp[:])
                nc.vector.tensor_scalar_mul(probs[:], probs[:], recip[:, 0:1])
                mx8 = gpool.tile([P, 8], f32, tag="mx8")
                nc.vector.max(mx8[:], probs[:])
                return probs, mx8

            # ---- local fp32 gate for range 0 (permuted: own expert col 0)
            wgtq0 = wgpool.tile([P, E], f32, tag="wgtq0")
            for g in range(2):
                ps_sT = psmall.tile([E, TPP], f32, tag="sm")
                for dk in range(DK):
                    nc.tensor.matmul(
                        ps_sT[:], lhsT=gwTp_s[:, dk, :], rhs=xgp[g][:, dk, :],
                        start=(dk == 0), stop=(dk == DK - 1))
                sT_sb = gpool.tile([E, TPP], f32, tag="sTsb")
                nc.vector.tensor_copy(sT_sb[:], ps_sT[:])
                for tt in range(TPP // P):
                    f = g * (TPP // P) + tt
                    ps_g = psmall.tile([P, E], f32, tag="sm")
                    nc.tensor.transpose(
                        ps_g[:], sT_sb[:, tt * P:(tt + 1) * P],
                        identf_s[:E, :E])
                    probs, mx8 = softmax_top2(ps_g)
                    ge = gpool.tile([P, 1], f32, tag="ge")
                    nc.vector.tensor_tensor(
                        ge[:], probs[:, 0:1], mx8[:, 1:2],
                        mybir.AluOpType.is_ge)
                    nc.vector.tensor_mul(
                        wgtq0[:, f:f + 1], probs[:, 0:1], ge[:])

            # ---- sharded fp32 gate on own 512 tokens (natural order) ----
            ps_sT = psmall.tile([E, TPP], f32, tag="sm")
            for dk in range(DK):
                nc.tensor.matmul(
                    ps_sT[:], lhsT=gwTn_s[:, dk, :], rhs=xgp[2][:, dk, :],
                    start=(dk == 0), stop=(dk == DK - 1))
            sT_sb = gpool.tile([E, TPP], f32, tag="sTsb")
            nc.vector.tensor_copy(sT_sb[:], ps_sT[:])
            wg_loc = wgpool.tile([P, TPP // P, E], f32, tag="wgloc")
            for tt in range(TPP // P):
                ps_g = psmall.tile([P, E], f32, tag="sm")
                nc.tensor.transpose(
                    ps_g[:], sT_sb[:, tt * P:(tt + 1) * P], identf_s[:E, :E])
                probs, mx8 = softmax_top2(ps_g)
                ge8 = gpool.tile([P, E], f32, tag="ge8")
                nc.vector.tensor_tensor(
                    ge8[:], probs[:], mx8[:, 1:2].to_broadcast([P, E]),
                    mybir.AluOpType.is_ge)
                nc.vector.tensor_mul(wg_loc[:, tt, :], probs[:], ge8[:])
            # back to expert-major [E, TPP] and publish
            wgT_loc = wgpool.tile([E, TPP], f32, tag="wgTloc")
            for tt in range(TPP // P):
                psb = psmall.tile([E, P], f32, tag="sm")
                nc.tensor.transpose(psb[:], wg_loc[:, tt, :], identf_s[:])
                nc.vector.tensor_copy(wgT_loc[:, tt * P:(tt + 1) * P], psb[:])
            wgl_d = dram.tile([E, TPP], f32, tag="wgl", name="wgl")
            nc.sync.dma_start(wgl_d[:, :], wgT_loc[:])

            wgallT_d = dram.tile([NCORES * E, TPP], f32, tag="wgall",
                                 name="wgall", addr_space="Shared")
            nc.gpsimd.collective_compute(
                "AllGather",
                mybir.AluOpType.bypass,
                replica_groups=[list(range(NCORES))],
                ins=[wgl_d[:, :].opt()],
                outs=[wgallT_d[:, :].opt()],
            )
            # indirect-gather rows (r'*E + my_e) of the allgathered [64, TPP]
            wgAT_s = wgpool.tile([E, TPP], f32, tag="wgAT")
            nc.gpsimd.indirect_dma_start(
                out=wgAT_s[:],
                out_offset=None,
                in_=wgallT_d[:, :],
                in_offset=bass.IndirectOffsetOnAxis(
                    ap=selidx_s[:, 0:1], axis=0))
            # token-partitions: wgt32v[:, tt, j] = weight of token
            # j*TPP + tt*P + p  (chunk index c = 4*j + tt)
            wgt32v = wgpool.tile([P, TPP // P, E], f32, tag="wgt32")
            for tt in range(TPP // P):
                pst = psmall.tile([P, E], f32, tag="sm")
                nc.tensor.transpose(
                    pst[:], wgAT_s[:, tt * P:(tt + 1) * P], identf_s[:E, :E])
                nc.vector.tensor_copy(wgt32v[:, tt, :], pst[:])

            # ===== phase B + C, software-pipelined over ranges =====
            def compact(q):
                """Prefix-sum slot assignment + fp16 one-hot list build,
                all on PE/DVE.  Returns (lst, gidx_i, yidx_i)."""
                if q == 0:
                    wq = wgtq0[:]        # natural chunk order
                    tid = tidb_s[:, 0, :]
                else:
                    wq = wgt32v[:, :, 2 * q:2 * q + 2]   # CHORD order
                    tid = tidb_s[:, 1, :]
                m = cpool.tile([P, E], f16, tag="m", name=f"m{q}")
                nc.vector.tensor_scalar(
                    m[:], wq, 0.0, scalar2=None, op0=mybir.AluOpType.is_gt)
                psA = psmall.tile([P, E], f32, tag="sm")
                nc.tensor.matmul(psA[:], lhsT=utri_s[:], rhs=m[:],
                                 start=True, stop=True)
                psC = psmall.tile([P, E], f32, tag="sm")
                nc.tensor.matmul(psC[:], lhsT=ones_s[:], rhs=m[:],
                                 start=True, stop=True)
                pos = cpool.tile([P, E], f32, tag="pos", name=f"pos{q}")
                nc.vector.tensor_copy(pos[:], psA[:])
                ctot = cpool.tile([P, E], f32, tag="ctot", name=f"ct{q}")
                nc.vector.tensor_copy(ctot[:], psC[:])
                for f in range(1, E):
                    nc.vector.tensor_add(
                        ctot[:, f:f + 1], ctot[:, f:f + 1], ctot[:, f - 1:f])
                for f in range(1, E):
                    nc.vector.tensor_add(
                        pos[:, f:f + 1], pos[:, f:f + 1], ctot[:, f - 1:f])
                # non-routed tokens park at pos = RT (matches no slot)
                nc.vector.tensor_scalar_add(pos[:], pos[:], float(-RT))
                nc.vector.tensor_mul(pos[:], pos[:], m[:])
                nc.vector.tensor_scalar_add(pos[:], pos[:], float(RT))

                # fp16 payload: [tid_local (<=1023, exact), wgt, occ]
                pay = cpool.tile([P, E, 3], f16, tag="pay", name=f"pay{q}")
                nc.vector.tensor_copy(pay[:, :, 0], tid)
                nc.vector.tensor_copy(pay[:, :, 1], wq)
                nc.vector.tensor_copy(pay[:, :, 2], m[:])

                lst = cpool.tile([P, CT, 3], f32, tag="lst", name=f"lst{q}")
                for ct in range(CT):
                    ps_l = psmall.tile([P, 3], f32, tag="sm")
                    for f in range(E):
                        ind = cpool.tile([P, P], f16, tag="ind")
                        nc.vector.tensor_tensor(
                            ind[:], pos[:, f:f + 1].to_broadcast([P, P]),
                            sr_s[:, ct * P:(ct + 1) * P],
                            mybir.AluOpType.is_equal)
                        nc.tensor.matmul(
                            ps_l[:], lhsT=ind[:], rhs=pay[:, f, :],
                            start=(f == 0), stop=(f == E - 1))
                    nc.vector.tensor_copy(lst[:, ct, :], ps_l[:])

                # pads (occ=0): gather the zero x row; scatter is dropped
                # by the bounds check (yidx = RT + tid_local >= RT)
                occ1 = cpool.tile([P, CT], f32, tag="occ1", name=f"occ1{q}")
                nc.vector.tensor_scalar(
                    occ1[:], lst[:, :, 2], -1.0, None,
                    op0=mybir.AluOpType.add)        # occ-1  (0 or -1)
                gidxf = cpool.tile([P, CT], f32, tag="gxf", name=f"gxf{q}")
                nc.vector.tensor_scalar(
                    gidxf[:], occ1[:], -float(T - q * RT), None,
                    op0=mybir.AluOpType.mult)       # (1-occ)*(T - q*RT)
                nc.vector.tensor_add(gidxf[:], gidxf[:], lst[:, :, 0])
                nc.vector.tensor_scalar_add(
                    gidxf[:], gidxf[:], float(q * RT))
                gidx_i = cpool.tile([P, CT], i32, tag="gidx", name=f"gi{q}")
                nc.vector.tensor_copy(gidx_i[:], gidxf[:])
                yidxf = cpool.tile([P, CT], f32, tag="yxf", name=f"yxf{q}")
                nc.vector.tensor_scalar(
                    yidxf[:], occ1[:], -float(RT), None,
                    op0=mybir.AluOpType.mult)       # (1-occ)*RT
                nc.vector.tensor_add(yidxf[:], yidxf[:], lst[:, :, 0])
                yidx_i = cpool.tile([P, CT], i32, tag="yidxi", name=f"yi{q}")
                nc.vector.tensor_copy(yidx_i[:], yidxf[:])
                return lst, gidx_i, yidx_i

            def gather_transpose(q, gidx_i):
                """Per c-tile indirect x-row gathers + PE transposes."""
                xeT = xtpool.tile([P, DK, CAP], bf16, tag="xeT",
                                  name=f"xeT{q}")
                for ct in range(CT):
                    rows = _ct_rows(ct)
                    xe = xepool.tile([P, D], bf16, tag="xe")
                    nc.gpsimd.indirect_dma_start(
                        out=xe[0:rows, :],
                        out_offset=None,
                        in_=x_d[:, :],
                        in_offset=bass.IndirectOffsetOnAxis(
                            ap=gidx_i[0:rows, ct:ct + 1], axis=0))
                    for dk in range(DK):
                        ptr = psmall.tile([P, P], bf16, tag="sm")
                        nc.tensor.transpose(
                            ptr[0:P, 0:rows],
                            xe[0:rows, dk * P:(dk + 1) * P],
                            identb_s[0:rows, 0:rows])
                        nc.scalar.copy(
                            xeT[:, dk, ct * P:ct * P + rows], ptr[:, 0:rows])
                return xeT

            state = {0: compact(0)}
            xeT_cur = gather_transpose(0, state[0][1])

            for q in range(NQ):
                lst, gidx_i, yidx_i = state.pop(q)
                if q + 1 < NQ:
                    state[q + 1] = compact(q + 1)

                aT = apool.tile([P, IK, CAP], bf16, tag="aT")
                for ik in range(IK):
                    isl = slice(ik * P, (ik + 1) * P)
                    ph = psum.tile([P, CAP], f32, tag="ph")
                    for dk in range(DK):
                        nc.tensor.matmul(
                            ph[:], lhsT=w1T_s[:, dk, isl],
                            rhs=xeT_cur[:, dk, :],
                            start=(dk == 0), stop=(dk == DK - 1))
                    pg = psum.tile([P, CAP], f32, tag="pg")
                    for dk in range(DK):
                        nc.tensor.matmul(
                            pg[:], lhsT=w3T_s[:, dk, isl],
                            rhs=xeT_cur[:, dk, :],
                            start=(dk == 0), stop=(dk == DK - 1))
                    sil = spool.tile([P, CAP], bf16, tag="sil")
                    nc.scalar.activation(
                        sil[:], ph[:], mybir.ActivationFunctionType.Silu)
                    nc.vector.tensor_mul(aT[:, ik, :], sil[:], pg[:])

                # next range's gather + transposes overlap this range's tail
                if q + 1 < NQ:
                    xeT_next = gather_transpose(q + 1, state[q + 1][1])

                for ct in range(CT):
                    rows = _ct_rows(ct)
                    yt = ypool.tile([P, D], bf16, tag="yt")
                    for dc in range(2):
                        py = pyps.tile([P, D // 2], f32, tag="py")
                        for ik in range(IK):
                            nc.tensor.matmul(
                                py[0:rows, :],
                                lhsT=aT[:, ik, ct * P:ct * P + rows],
                                rhs=w2T_s[:, ik,
                                          dc * (D // 2):(dc + 1) * (D // 2)],
                                start=(ik == 0), stop=(ik == IK - 1))
                        nc.vector.tensor_scalar_mul(
                            yt[0:rows, dc * (D // 2):(dc + 1) * (D // 2)],
                            py[0:rows, :], lst[0:rows, ct, 1:2])
                    # pad rows (yidx >= RT) are dropped by the bounds check
                    nc.gpsimd.indirect_dma_start(
                        out=ycontribs[q][:, :],
                        out_offset=bass.IndirectOffsetOnAxis(
                            ap=yidx_i[0:rows, ct:ct + 1], axis=0),
                        in_=yt[0:rows, :],
                        in_offset=None,
                        bounds_check=RT - 1,
                        oob_is_err=False)

                nc.gpsimd.collective_compute(
                    "ReduceScatter",
                    mybir.AluOpType.add,
                    replica_groups=[list(range(NCORES))],
                    ins=[ycontribs[q][:, :].opt()],
                    outs=[yshards[q].opt()],
                )
                if q + 1 < NQ:
                    xeT_cur = xeT_next

            # ship shards at the very end: a mid-pipeline y write would
            # make later ranges' work wait on the collective
            for q in range(NQ):
                nc.sync.dma_start(y_d[q * RSH:(q + 1) * RSH, :], yshards[q][:])
    nc.compile()
    return nc


def _get_nc():
    global _CACHED_NC
    if _CACHED_NC is None:
        _CACHED_NC = _build()
    return _CACHED_NC


def _preshuffle(mat, nk):
    """[nk*P, M] -> [P, nk*M] with row p = concat_o mat[o*P + p, :]."""
    nkP, M = mat.shape
    assert nkP == nk * P
    return np.ascontiguousarray(
        mat.reshape(nk, P, M).transpose(1, 0, 2).reshape(P, nk * M))


def _in_maps(x, gate_w, w1, w3, w2):
    x = np.asarray(x, dtype=np.float32)
    gate_w = np.asarray(gate_w, dtype=np.float32)
    xpad = np.zeros((XPAD_ROWS, D), dtype=BF)
    xpad[:T] = x.astype(BF)

    # host-side capacity check against the actual gate (cheap, exact)
    s = x @ gate_w.T
    thr = np.sort(s, axis=1)[:, -TOPK]          # 2nd-largest score
    routed = s >= thr[:, None]                  # [T, E]
    cnt = routed.reshape(NQ, RT, E).sum(axis=1)  # [NQ, E]
    if cnt.max() > CAP:
        raise RuntimeError(f"routing capacity exceeded: {cnt.max()} > {CAP}")

    F16 = np.float16
    utri = np.triu(np.ones((P, P), np.float32), k=1).astype(F16)
    ones = np.ones((P, P), F16)
    identf = np.eye(P, dtype=np.float32)
    identb = np.eye(P, dtype=np.float32).astype(BF)
    # tidb holds LOCAL (within-range) token bases, exact in fp16
    # slot 0: natural order (range 0); slot 1: CHORD order (AG ranges)
    tidb_nat = (np.arange(E, dtype=np.float32)[None, :] * P
                + np.arange(P, dtype=np.float32)[:, None])
    tidb_ch = (np.array(CHORD, np.float32)[None, :] * P
               + np.arange(P, dtype=np.float32)[:, None])
    tidb = np.concatenate([tidb_nat, tidb_ch], axis=1).astype(np.float32)
    sr = np.broadcast_to(np.arange(CT * P, dtype=np.float32)[None, :],
                         (P, CT * P)).copy()
    gwTn_pre = _preshuffle(np.ascontiguousarray(gate_w.T), DK)
    # per-512-token gate pieces, pre-shuffled [P, DK*TPP] each
    xT = np.ascontiguousarray(x.T)  # [D, T]
    pieces = [
        np.ascontiguousarray(
            xT[:, g * TPP:(g + 1) * TPP].reshape(DK, P, TPP)
            .transpose(1, 0, 2).reshape(P, DK * TPP))
        for g in range(NCORES)
    ]

    maps = []
    for e in range(NCORES):
        perm = [e] + [j for j in range(E) if j != e]
        gwTp_pre = _preshuffle(np.ascontiguousarray(gate_w[perm].T), DK)
        maps.append({
            "xgT": np.concatenate([pieces[0], pieces[1], pieces[e]], axis=1),
            "x": xpad,
            "gwTn": gwTn_pre,
            "gwTp": gwTp_pre,
            "w1T": _preshuffle(
                np.ascontiguousarray(np.asarray(w1[e], np.float32).T), DK
            ).astype(BF),
            "w3T": _preshuffle(
                np.ascontiguousarray(np.asarray(w3[e], np.float32).T), DK
            ).astype(BF),
            "w2T": _preshuffle(
                np.ascontiguousarray(np.asarray(w2[e], np.float32).T), IK
            ).astype(BF),
            "utri": utri,
            "ones": ones,
            "identf": identf,
            "identb": identb,
            "tidb": tidb,
            "sr": sr,
            "selidx": (np.arange(NCORES, dtype=np.int32)[:, None] * E
                       + np.int32(e)),
        })
    return maps


def run(x, gate_w, w1, w3, w2, trace=False, trace_cores=None):
    nc = _get_nc()
    maps = _in_maps(x, gate_w, w1, w3, w2)
    res = run_bass_kernel_spmd(
        nc, maps, core_ids=list(range(NCORES)), trace=trace,
        trace_cores=trace_cores)
    # core r's output block q (128 rows) holds tokens [1024q + 128r, +128)
    y = np.empty((T, D), dtype=np.float32)
    for r in range(NCORES):
        yr = np.asarray(res.results[r]["y"]).astype(np.float32)
        for q in range(NQ):
            t0 = q * RT + r * RSH
            y[t0:t0 + RSH] = yr[q * RSH:(q + 1) * RSH]
    return y, res


def kernel(x, gate_w, w1, w3, w2):
    y, _ = run(x, gate_w, w1, w3, w2, trace=False)
    return y.astype(np.float32)
